# revision 6
# baseline (speedup 1.0000x reference)
"""Trainium2 Bass kernel for MultiHeadAttention (B=2, S=2048, D=1024, H=16).

Sharding: 8 cores = 2 batches x 4 head-groups (4 heads each).
Each core computes QKV projections for its (batch, 4 heads), full causal
attention over seq, and a partial output projection (its 256 rows of Wo).
Host sums the 4 partials per batch and adds bo_eff = bo + bv @ Wo
(the V bias passes through softmax rows which sum to 1).

Returns (out, attn) matching reference.py.
"""

import sys

sys.path.insert(0, "/opt/trn_rl_repo")

from contextlib import ExitStack

import numpy as np

import concourse.bass as bass
import concourse.bacc as bacc
import concourse.mybir as mybir
import concourse.tile as tile
from concourse.bass_utils import run_bass_kernel_spmd
from concourse.masks import make_causal_mask, make_identity

B, S, D, H = 2, 2048, 1024, 16
DEPTH = D // H  # 64
HG = 4  # head-groups (cores per batch)
HPG = H // HG  # heads per core = 4
DG = HPG * DEPTH  # 256 d-columns per core
P = 128
NKT = D // P  # 8 contraction tiles for projections
NQT = S // P  # 16 query tiles
SCALE = 1.0 / float(np.sqrt(DEPTH))  # 0.125
NEG = -1.0e17
F32 = mybir.dt.float32

_nc_cache = {}


def _build_nc():
    if "nc" in _nc_cache:
        return _nc_cache["nc"]
    nc = bacc.Bacc()

    xq_t = nc.declare_dram_parameter("xq_t", [D, S], F32, isOutput=False)
    xk_t = nc.declare_dram_parameter("xk_t", [D, S], F32, isOutput=False)
    xv_t = nc.declare_dram_parameter("xv_t", [D, S], F32, isOutput=False)
    wq = nc.declare_dram_parameter("wq", [D, DG], F32, isOutput=False)
    wk = nc.declare_dram_parameter("wk", [D, DG], F32, isOutput=False)
    wv = nc.declare_dram_parameter("wv", [D, DG], F32, isOutput=False)
    wo = nc.declare_dram_parameter("wo", [DG, D], F32, isOutput=False)
    bq = nc.declare_dram_parameter("bq", [DG], F32, isOutput=False)
    bk = nc.declare_dram_parameter("bk", [DG], F32, isOutput=False)

    attn_out = nc.declare_dram_parameter("attn_out", [HPG, S, S], F32, isOutput=True)
    out_part = nc.declare_dram_parameter("out_part", [S, D], F32, isOutput=True)

    _emit(nc, xq_t, xk_t, xv_t, wq, wk, wv, wo, bq, bk, attn_out, out_part)
    nc.compile()
    _nc_cache["nc"] = nc
    return nc


def _emit(nc, xq_t, xk_t, xv_t, wq, wk, wv, wo, bq, bk, attn_out, out_part):
    with tile.TileContext(nc) as tc, ExitStack() as ctx:
        singles = ctx.enter_context(tc.tile_pool(name="singles", bufs=1))
        xfull = ctx.enter_context(tc.tile_pool(name="xfull", bufs=1))
        ps512 = ctx.enter_context(tc.tile_pool(name="ps512", bufs=2, space="PSUM"))
        ps1024 = ctx.enter_context(tc.tile_pool(name="ps1024", bufs=2, space="PSUM"))
        ps128 = ctx.enter_context(tc.tile_pool(name="ps128", bufs=2, space="PSUM"))
        attn_pool = ctx.enter_context(tc.tile_pool(name="attn", bufs=2))
        att_t = ctx.enter_context(tc.tile_pool(name="att_t", bufs=2))
        outp_pool = ctx.enter_context(tc.tile_pool(name="outp", bufs=2))
        stats = ctx.enter_context(tc.tile_pool(name="stats", bufs=4))
        vacc = ctx.enter_context(tc.tile_pool(name="vacc", bufs=2))

        # --- constants -------------------------------------------------
        identity = singles.tile([P, P], F32)
        make_identity(nc, identity)
        cmask = singles.tile([P, P], F32)
        make_causal_mask(nc, cmask, mask_val=NEG)
        zero_t = singles.tile([P, 1024], F32)
        nc.vector.memset(zero_t, 0.0)

        # --- weights ---------------------------------------------------
        wq_sb = singles.tile([P, NKT, DG], F32)
        wk_sb = singles.tile([P, NKT, DG], F32)
        wv_sb = singles.tile([P, NKT, DG], F32)
        nc.sync.dma_start(out=wq_sb, in_=wq.rearrange("(kt p) m -> p kt m", p=P))
        nc.sync.dma_start(out=wk_sb, in_=wk.rearrange("(kt p) m -> p kt m", p=P))
        nc.sync.dma_start(out=wv_sb, in_=wv.rearrange("(kt p) m -> p kt m", p=P))
        wo_sb = singles.tile([P, 2, D], F32)
        nc.sync.dma_start(out=wo_sb, in_=wo.rearrange("(c p) n -> p c n", p=P))
        bq_sb = singles.tile([P, 2], F32)
        bk_sb = singles.tile([P, 2], F32)
        nc.sync.dma_start(out=bq_sb, in_=bq.rearrange("(c p) -> p c", p=P))
        nc.sync.dma_start(out=bk_sb, in_=bk.rearrange("(c p) -> p c", p=P))

        # --- persistent activations -----------------------------------
        # QT/KT: [p, c, s] = projected^T, d-index = c*128+p (head h at
        # chunk h//2, partitions (h%2)*64 ..)
        QT = singles.tile([P, 2, S], F32)
        KT = singles.tile([P, 2, S], F32)
        # V: [p, st, m] = V[st*128+p, m], m = local d column (head h*64+dd)
        V_sb = singles.tile([P, NQT, DG], F32)
        # out_catT: [p, c, q]: rows hd = c*128+p (head-pair c)
        ocT = singles.tile([P, 2, S], F32)

        # --- V projection (x_v fully resident, st-outer kt-inner) ------
        xv_sb = xfull.tile([P, NKT, S], F32, tag="x")
        nc.sync.dma_start(out=xv_sb, in_=xv_t.rearrange("(kt p) s -> p kt s", p=P))
        for st in range(NQT):
            ps = ps512.tile([P, DG], F32, tag="ps512")
            for kt in range(NKT):
                nc.tensor.matmul(
                    ps,
                    lhsT=xv_sb[:, kt, st * P : (st + 1) * P],
                    rhs=wv_sb[:, kt, :],
                    start=(kt == 0),
                    stop=(kt == NKT - 1),
                )
            nc.vector.tensor_copy(V_sb[:, st, :], ps)

        # --- Q/K projections --------------------------------------------
        for x_dram, w_sb, b_sb, dst in (
            (xq_t, wq_sb, bq_sb, QT),
            (xk_t, wk_sb, bk_sb, KT),
        ):
            x_sb = xfull.tile([P, NKT, S], F32, tag="x")
            nc.sync.dma_start(out=x_sb, in_=x_dram.rearrange("(kt p) s -> p kt s", p=P))
            for c in range(2):
                for sq in range(4):
                    ps = ps512.tile([P, 512], F32, tag="ps512")
                    for kt in range(NKT):
                        nc.tensor.matmul(
                            ps,
                            lhsT=w_sb[:, kt, c * P : (c + 1) * P],
                            rhs=x_sb[:, kt, sq * 512 : (sq + 1) * 512],
                            start=(kt == 0),
                            stop=(kt == NKT - 1),
                        )
                    nc.vector.tensor_scalar_add(
                        dst[:, c, sq * 512 : (sq + 1) * 512], ps, b_sb[:, c : c + 1]
                    )

        _emit_attention(
            nc, tc, singles, ps512, ps1024, ps128, attn_pool, att_t, outp_pool,
            stats, identity, cmask, zero_t, QT, KT, V_sb, ocT, wo_sb,
            attn_out, out_part,
        )


def _emit_attention(
    nc, tc, singles, ps512, ps1024, ps128, attn_pool, att_t, outp_pool,
    stats, identity, cmask, zero_t, QT, KT, V_sb, ocT, wo_sb, attn_out, out_part,
):
    zbias = singles.tile([P, 1], F32)
    nc.vector.memset(zbias, 0.0)

    for hp in range(2):
        for qt in range(NQT):
            row = (qt + 1) * P
            nparts = (row + 1023) // 1024
            o_ps = ps128.tile([P, P], F32, tag="ops")
            for e in range(2):
                h = hp * 2 + e
                c = h // 2
                pb = (h % 2) * 64
                qh = QT[pb : pb + 64, c, :]
                kh = KT[pb : pb + 64, c, :]

                # logits (raw qk) into psum parts of <=1024 cols
                parts = []
                for ip in range(nparts):
                    width = min(1024, row - ip * 1024)
                    lp = ps1024.tile([P, 1024], F32, tag="lps")
                    parts.append((lp, width))
                    for kc in range((width + 511) // 512):
                        w = min(512, width - kc * 512)
                        nc.tensor.matmul(
                            lp[:, kc * 512 : kc * 512 + w],
                            lhsT=qh[:, qt * P : (qt + 1) * P],
                            rhs=kh[:, ip * 1024 + kc * 512 : ip * 1024 + kc * 512 + w],
                            start=True,
                            stop=True,
                        )
                # causal mask on the diagonal 128x128 block
                dlp, _ = parts[-1]
                doff = qt * P - (nparts - 1) * 1024
                nc.vector.tensor_add(
                    dlp[:, doff : doff + P], dlp[:, doff : doff + P], cmask
                )
                # exp + row-sums
                attn_sb = attn_pool.tile([P, S], F32, tag="attn")
                sums = stats.tile([P, 2], F32, tag="sums")
                for ip, (lp, width) in enumerate(parts):
                    nc.scalar.activation(
                        out=attn_sb[:, ip * 1024 : ip * 1024 + width],
                        in_=lp[:, :width],
                        func=mybir.ActivationFunctionType.Exp,
                        bias=zbias,
                        scale=SCALE,
                        accum_out=sums[:, ip : ip + 1],
                    )
                inv = stats.tile([P, 1], F32, tag="inv")
                if nparts == 2:
                    nc.vector.tensor_add(sums[:, 0:1], sums[:, 0:1], sums[:, 1:2])
                nc.vector.reciprocal(inv, sums[:, 0:1])
                nc.vector.tensor_scalar_mul(attn_sb[:, :row], attn_sb[:, :row], inv)

                # DMA attn row block out (computed part + zero tail)
                nc.sync.dma_start(
                    out=attn_out[h, qt * P : (qt + 1) * P, 0:row],
                    in_=attn_sb[:, :row],
                )
                tail = S - row
                toff = row
                while tail > 0:
                    w = min(1024, tail)
                    nc.sync.dma_start(
                        out=attn_out[h, qt * P : (qt + 1) * P, toff : toff + w],
                        in_=zero_t[:, :w],
                    )
                    tail -= w
                    toff += w

                # transpose attn tiles (batches of 4) + attn @ V
                for kt0 in range(0, qt + 1, 4):
                    nbt = min(4, qt + 1 - kt0)
                    tp = ps512.tile([P, 512], F32, tag="ps512")
                    for j in range(nbt):
                        kt = kt0 + j
                        nc.tensor.transpose(
                            tp[:, j * P : (j + 1) * P],
                            attn_sb[:, kt * P : (kt + 1) * P],
                            identity,
                        )
                    at = att_t.tile([P, 512], F32, tag="att_t")
                    nc.vector.tensor_copy(at[:, : nbt * P], tp[:, : nbt * P])
                    for j in range(nbt):
                        kt = kt0 + j
                        nc.tensor.matmul(
                            o_ps[pb : pb + 64, :],
                            lhsT=V_sb[:, kt, h * DEPTH : (h + 1) * DEPTH],
                            rhs=at[:, j * P : (j + 1) * P],
                            start=(kt == 0),
                            stop=(kt == qt),
                            tile_position=(0, pb),
                        )
            nc.vector.tensor_copy(ocT[:, hp, qt * P : (qt + 1) * P], o_ps)

    # --- output projection ---------------------------------------------
    for qt in range(NQT):
        fp = ps1024.tile([P, 1024], F32, tag="lps")
        for c in range(2):
            for nck in range(2):
                nc.tensor.matmul(
                    fp[:, nck * 512 : (nck + 1) * 512],
                    lhsT=ocT[:, c, qt * P : (qt + 1) * P],
                    rhs=wo_sb[:, c, nck * 512 : (nck + 1) * 512],
                    start=(c == 0),
                    stop=(c == 1),
                )
        op = outp_pool.tile([P, D], F32, tag="outp")
        nc.vector.tensor_copy(op, fp)
        nc.sync.dma_start(out=out_part[qt * P : (qt + 1) * P, :], in_=op)


def kernel(**inputs):
    v = np.asarray(inputs["v"], np.float32)
    k = np.asarray(inputs["k"], np.float32)
    q = np.asarray(inputs["q"], np.float32)
    Wq = np.asarray(inputs["Wq"], np.float32)
    Wk = np.asarray(inputs["Wk"], np.float32)
    Wv = np.asarray(inputs["Wv"], np.float32)
    Wo = np.asarray(inputs["Wo"], np.float32)
    bq = np.asarray(inputs["bq"], np.float32)
    bk = np.asarray(inputs["bk"], np.float32)
    bv = np.asarray(inputs["bv"], np.float32)
    bo = np.asarray(inputs["bo"], np.float32)

    nc = _build_nc()

    xT = {}
    for b in range(B):
        xT[("q", b)] = np.ascontiguousarray(q[b].T)
        xT[("k", b)] = np.ascontiguousarray(k[b].T)
        xT[("v", b)] = np.ascontiguousarray(v[b].T)

    in_maps = []
    for core in range(8):
        b, hg = core // HG, core % HG
        cols = slice(hg * DG, (hg + 1) * DG)
        in_maps.append(
            {
                "xq_t": xT[("q", b)],
                "xk_t": xT[("k", b)],
                "xv_t": xT[("v", b)],
                "wq": np.ascontiguousarray(Wq[:, cols]),
                "wk": np.ascontiguousarray(Wk[:, cols]),
                "wv": np.ascontiguousarray(Wv[:, cols]),
                "wo": np.ascontiguousarray(Wo[cols, :]),
                "bq": np.ascontiguousarray(bq[cols]),
                "bk": np.ascontiguousarray(bk[cols]),
            }
        )

    res = run_bass_kernel_spmd(nc, in_maps, list(range(8))).results

    bo_eff = bo + bv @ Wo
    out = np.zeros((B, S, D), np.float32)
    attn = np.empty((B, H, S, S), np.float32)
    for core in range(8):
        b, hg = core // HG, core % HG
        attn[b, hg * HPG : (hg + 1) * HPG] = res[core]["attn_out"]
        out[b] += res[core]["out_part"]
    out += bo_eff[None, None, :]
    return out, attn
